# revision 26
# baseline (speedup 1.0000x reference)
"""Always-on MoE forward (expert 0 dense + top-k of 7 routed) on 8 TRN2 cores.

Strategy
--------
The router (4096x1024 @ 1024x7 matmul + softmax + top-2) is ~58 MFLOP --
negligible -- so it runs on host in numpy as part of computing the sharding
plan.  The expensive part (expert SwiGLU MLPs, ~155 GFLOP with top-2
sparsity) runs on device, expert-parallel with host-side token
dispatch/combine:

- SPMD graph: every core runs TWO weight groups with compile-time capacities
  (A, B).  Each (core, group) slot is filled with tokens of ONE expert
  (weights supplied per-core via in_maps).  The 7 routed experts are packed
  into the 8 B-slots (splitting an expert across slots when that lowers the
  makespan); leftover B-slots and all A-slots are filled with always-on
  expert-0 tokens.  (A, B) are optimized per call from the actual router
  counts, so per-core capacity lands within a few % of the ideal
  (T + k*T) / 8 balance point.
- Host gathers each core's tokens (transposed k-chunk layout, bf16), device
  computes down(silu(x@wg) * (x@wu)) for both groups, host scatter-adds the
  outputs with the combine weights (expert-0 weight 1.0).

Device kernel: pure dense matmul streaming, weights stationary in SBUF per
group.  All DRAM views are pre-tiled on host into [128, flat] layouts so
every DMA is contiguous per partition and every matmul operand is a direct
SBUF slice.

DMA schedule (measured on HW, see comments in _build_graph): geometric
[1,1,2,4,8]-m-tile weight blocks in exact consumption order across the two
HWDGE rings -- sync carries wg + the first two wu blocks + wd, scalar
carries xt (first tile k-chunked) + the remaining wu + outputs.  A ~4 us
PE pre-warm burst flips the HAM clock gate during the DMA ramp so payload
matmuls start warm the moment wg block0 lands (~12 us).  Keep per-ring
dma_start counts low: each dma_start costs its engine ~0.6-1.2 us and only
~6 can be outstanding; finer-grained schedules measured SLOWER (engine
issue serialization starves the ring mid-stream), and the gpsimd SWDGE
queue is far too slow for bulk tiles (+65 us when tried).
"""

import numpy as np
import ml_dtypes

D = 1024
DFF = 2048
E = 8
NCORES = 8
T = 2 * 2048  # B * S
KD = D // 128    # contraction chunks over D
KF = DFF // 128  # contraction chunks over DFF

_COMPILED = {}

_BF16 = ml_dtypes.bfloat16

LAST_EXEC_NS = None


def _route(x, router_w, router_b, top_k):
    """Replicates the reference router in numpy f32: returns (topi, topw)."""
    logits = x.astype(np.float32) @ router_w.astype(np.float32) + router_b.astype(
        np.float32
    )
    m = logits.max(axis=-1, keepdims=True)
    p = np.exp(logits - m)
    p /= p.sum(axis=-1, keepdims=True)
    k = int(top_k)
    topi = np.argpartition(-p, kth=k - 1, axis=-1)[:, :k]  # top-k set (unordered)
    topw = np.take_along_axis(p, topi, axis=-1)
    topw = topw / topw.sum(axis=-1, keepdims=True)
    return topi, topw.astype(np.float32)


def _split_even(idx, w, n):
    """Split (idx, w) into n near-equal chunks."""
    c = len(idx)
    sizes = [(c + n - 1 - i) // n for i in range(n)]
    out, pos = [], 0
    for s in sizes:
        out.append((idx[pos : pos + s], w[pos : pos + s]))
        pos += s
    return out


def _widths_for(cap):
    """Tile widths for a group capacity.

    First tile as wide as possible (512) -- during the first tile the weight
    stream runs near the HBM rate, and a wider tile lowers the per-ns weight
    demand.  Remaining capacity in near-equal tiles, kept above the ~280
    matmul instruction floor (LDWEIGHTS + dispatch) when possible.
    """
    if cap <= 0:
        return []

    def near_equal(c, n):
        return [(c + n - 1 - i) // n for i in range(n)]

    plain = near_equal(cap, -(-cap // 512))
    if cap > 512:
        rem = cap - 512
        lead = [512] + near_equal(rem, -(-rem // 512))
    else:
        lead = plain
    cost = lambda ws: sum(max(w, 280) for w in ws)
    return lead if cost(lead) <= cost(plain) else plain


def _tile_cost(cap):
    return sum(max(w, 280) for w in _widths_for(cap))


def _plan_slots(per_expert):
    """Pick capacities (A, B) and fill 8 A-slots + 8 B-slots.

    per_expert: {e: (idx, w)} for routed experts.
    Returns (A, B, slots) with slots[c] = ((eA, idxA, wA), (eB, idxB, wB)).
    """
    counts = {e: len(v[0]) for e, v in per_expert.items()}
    cands = sorted(
        {-(-c // j) for c in counts.values() for j in range(1, 9) if c} | {512}
    )
    best = None
    for Bc in cands:
        npieces = sum(-(-c // Bc) for c in counts.values() if c)
        if npieces > NCORES:
            continue
        nfree = NCORES - npieces
        e0_in_b = min(T, nfree * Bc)
        A = -(-(T - e0_in_b) // NCORES) if e0_in_b < T else 0
        tot = _tile_cost(A) + _tile_cost(Bc)
        if best is None or tot < best[0]:
            best = (tot, A, Bc)
    _, A, B = best

    ones = np.ones(T, dtype=np.float32)
    e0_idx = np.arange(T, dtype=np.int64)

    b_slots = []
    for e in sorted(counts, key=lambda e: -counts[e]):
        idx, w = per_expert[e]
        if len(idx) == 0:
            continue
        n = -(-len(idx) // B)
        for piece in _split_even(idx, w, n):
            b_slots.append((e, piece[0], piece[1]))
    pos = 0
    while len(b_slots) < NCORES:
        take = min(B, T - pos)
        b_slots.append((0, e0_idx[pos : pos + take], ones[pos : pos + take]))
        pos += take
    rem_idx = e0_idx[pos:]
    a_pieces = _split_even(rem_idx, ones[pos:], NCORES)
    a_slots = [(0, p[0], p[1]) for p in a_pieces]

    slots = [(a_slots[c], b_slots[c]) for c in range(NCORES)]
    return A, B, slots


def _wgu_layout(w2d):
    """[D, DFF] f32 -> [128, KF*KD*128] bf16 m-tile-major layout:
    element [p, (m*KD + k)*128 + c] = W[k*128 + p, m*128 + c]."""
    a = w2d.reshape(KD, 128, KF, 128).transpose(1, 2, 0, 3).reshape(128, -1)
    return np.ascontiguousarray(a).astype(_BF16)


def _wd_layout(w2d):
    """[DFF, D] f32 -> [128, KD*KF*128] bf16 m-tile-major layout:
    element [p, (m*KF + k)*128 + c] = W[k*128 + p, m*128 + c]."""
    a = w2d.reshape(KF, 128, KD, 128).transpose(1, 2, 0, 3).reshape(128, -1)
    return np.ascontiguousarray(a).astype(_BF16)


def _tiles_for(A, B):
    """Token tiles [(group, start_in_group, width, flat_offset)] per core."""
    tiles = []
    off = 0
    for g, cap in enumerate([A, B]):
        ts = 0
        for w in _widths_for(cap):
            tiles.append((g, ts, w, off))
            off += w
            ts += w
    return tiles, off


def _build_graph(A, B):
    import concourse.mybir as mybir
    import concourse.tile as tile
    from concourse import bacc
    from contextlib import ExitStack

    bf16 = mybir.dt.bfloat16
    f32 = mybir.dt.float32

    tiles, total = _tiles_for(A, B)
    groups = [g for g in range(2) if (A, B)[g] > 0]

    nc = bacc.Bacc("TRN2", target_bir_lowering=False)

    xt_d = nc.declare_dram_parameter("xt", [128, KD * total], bf16, isOutput=False)
    w_ds = []
    for g in groups:
        w_ds.append(
            (
                nc.declare_dram_parameter(
                    f"w{g}g", [128, KF * KD * 128], bf16, isOutput=False
                ),
                nc.declare_dram_parameter(
                    f"w{g}u", [128, KF * KD * 128], bf16, isOutput=False
                ),
                nc.declare_dram_parameter(
                    f"w{g}d", [128, KD * KF * 128], bf16, isOutput=False
                ),
            )
        )
    out_d = nc.declare_dram_parameter("out", [128, KD * total], bf16, isOutput=True)

    with tile.TileContext(nc) as tc, ExitStack() as ctx:
        wpool = ctx.enter_context(tc.tile_pool(name="weights", bufs=1))
        xpool = ctx.enter_context(tc.tile_pool(name="x", bufs=3))
        hpool = ctx.enter_context(tc.tile_pool(name="h", bufs=2))
        gpool = ctx.enter_context(tc.tile_pool(name="gact", bufs=3))
        opool = ctx.enter_context(tc.tile_pool(name="o", bufs=2))
        psg = ctx.enter_context(tc.tile_pool(name="psg", bufs=2, space="PSUM"))
        psu = ctx.enter_context(tc.tile_pool(name="psu", bufs=2, space="PSUM"))
        psd = ctx.enter_context(tc.tile_pool(name="psd", bufs=2, space="PSUM"))

        GU_M = KD * 128  # bytes-per-m-tile span (elems) for wg/wu
        D_M = KF * 128   # for wd

        # PE pre-warm: dummy matmuls run while the first weight/token DMAs are
        # in flight, flipping the HAM clock gate to 8/8 (2.4 GHz) before the
        # first real matmul lands.  Sized to end right as the first payload
        # matmul's inputs (wg block0 + xt0 chunk0) land.
        warm_sb = wpool.tile([128, 128], bf16, tag="warm")
        nc.vector.memset(warm_sb[:], 0)
        psw = ctx.enter_context(tc.tile_pool(name="psw", bufs=1, space="PSUM"))
        ps_w = psw.tile([128, 128], f32, tag="psw")
        for _ in range(38):
            nc.tensor.matmul(ps_w[:], warm_sb[:], warm_sb[:], start=True, stop=True)

        # Larger group first: its weights stream in unblocked at t=0, and the
        # other group's weight reloads (WAR-gated on this group's last use of
        # each m-tile slot) get a long compute window to hide under.
        order = sorted(range(len(groups)), key=lambda gi: -((A, B)[groups[gi]]))

        # DMA schedule, tuned against measured ring behavior: ~150-185 GB/s
        # per HWDGE ring when both flow, ~2 us sem-receipt lag per block,
        # each dma_start costs its engine ~0.6-1.2 us of issue time and only
        # ~6 may be outstanding per engine -- so blocks must stay FEW (the
        # geometric [1,1,2,4,8] m-tile split) and in consumption order.
        #   sync ring:   wg_b0, wu_b0, wg_b1, wu_b1, wg_b2..b4, wd_b0..b2
        #   scalar ring: xt0 in 4 k-chunks (first matmul needs only 0.25 MB),
        #                wu_b2..b4, remaining xt tiles, down-phase out DMAs.
        # wd is first needed ~55 us after payload start, so it tails sync.
        GU_BLKS = [(0, 1), (1, 2), (2, 4), (4, 8), (8, 16)]   # m-tile ranges
        WD_BLKS = [(0, 2), (2, 4), (4, 8)]                    # m2-tile ranges

        for gi in order:
            g = groups[gi]
            wg_d, wu_d, wd_d = w_ds[gi]
            first_group = gi == order[0]
            g_tiles = [(ts2, w_, off) for tg2, ts2, w_, off in tiles if tg2 == g]
            xt_sbs = {}
            ts0, w0, off0 = g_tiles[0]
            xt_sb0 = xpool.tile([128, KD * w0], bf16, tag="xt")
            src0 = xt_d.ap()[:, KD * off0 : KD * (off0 + w0)]
            if first_group:
                # xt0 in 4 k-chunks leading the scalar ring: the first
                # matmul needs only the first 0.25 MB.  (Tried and REJECTED,
                # all measured slower: finer front chunks + smaller wg_b0
                # head with warmup 32 -- payload starts earlier than the
                # stream can sustain, stalls move after the start and HAM
                # re-throttles; chunks/xt on gpsimd SWDGE -- +65 us; xt0
                # split across sync -- starves wg/wu.)
                for ci in range(4):
                    c0, c1 = 2 * ci * w0, 2 * (ci + 1) * w0
                    nc.scalar.dma_start(xt_sb0[:, c0:c1], src0[:, c0:c1])
            else:
                nc.scalar.dma_start(xt_sb0[:], src0)
            xt_sbs[ts0] = xt_sb0
            wg_blk, wu_blk, wd_blk = [], [], []
            for bi, (s, e) in enumerate(GU_BLKS):
                span = (e - s) * GU_M
                tg = wpool.tile([128, span], bf16, tag=f"wg_b{bi}")
                nc.sync.dma_start(tg[:], wg_d.ap()[:, s * GU_M : e * GU_M])
                wg_blk.append(tg)
                tu = wpool.tile([128, span], bf16, tag=f"wu_b{bi}")
                eng = nc.sync if bi < 2 else nc.scalar
                eng.dma_start(tu[:], wu_d.ap()[:, s * GU_M : e * GU_M])
                wu_blk.append(tu)
            # remaining xt tiles of this group (behind wu blocks on scalar;
            # needed only when their token tile starts, ~60+ us later)
            for ts2, w_, off in g_tiles[1:]:
                xt_sb = xpool.tile([128, KD * w_], bf16, tag="xt")
                nc.scalar.dma_start(
                    xt_sb[:], xt_d.ap()[:, KD * off : KD * (off + w_)]
                )
                xt_sbs[ts2] = xt_sb
            for bi, (s, e) in enumerate(WD_BLKS):
                span = (e - s) * D_M
                td = wpool.tile([128, span], bf16, tag=f"wd_b{bi}")
                nc.sync.dma_start(td[:], wd_d.ap()[:, s * D_M : e * D_M])
                wd_blk.append(td)

            def gu_slice(blk_list, m, k):
                for bi, (s, e) in enumerate(GU_BLKS):
                    if s <= m < e:
                        base = ((m - s) * KD + k) * 128
                        return blk_list[bi][:, base : base + 128]
                raise AssertionError

            def wd_slice(m2, k2):
                for bi, (s, e) in enumerate(WD_BLKS):
                    if s <= m2 < e:
                        base = ((m2 - s) * KF + k2) * 128
                        return wd_blk[bi][:, base : base + 128]
                raise AssertionError

            for tg_, ts, w, off in tiles:
                if tg_ != g:
                    continue
                rhs = lambda k, t=xt_sbs[ts]: t[:, k * w : k * w + w]
                h_sb = hpool.tile([128, KF, 512], bf16, tag="h")
                for m in range(KF):
                    ps_g = psg.tile([128, 512], f32, tag="psg")
                    ps_u = psu.tile([128, 512], f32, tag="psu")
                    for k in range(KD):
                        nc.tensor.matmul(
                            ps_g[:, :w],
                            gu_slice(wg_blk, m, k),
                            rhs(k),
                            start=(k == 0),
                            stop=(k == KD - 1),
                        )
                    if first_group and ts == 0 and m == 0:
                        # ~1.7-3 us of dummy matmuls (same shape/bank as the
                        # pre-warm burst -- do NOT resize psw, that shifts
                        # PSUM bank allocation and serializes the down
                        # phase): the up-m0/gate-m1 weight blocks land ~2-3
                        # us after this point, and an idle wait here can
                        # cluster past the 3.4 us HAM MID window and
                        # re-throttle the PE to 1.2 GHz (cold-MM train,
                        # ~3 us).  Busy-spinning instead keeps K=8/8.
                        for _ in range(30):
                            nc.tensor.matmul(
                                ps_w[:], warm_sb[:], warm_sb[:],
                                start=True, stop=True,
                            )
                    for k in range(KD):
                        nc.tensor.matmul(
                            ps_u[:, :w],
                            gu_slice(wu_blk, m, k),
                            rhs(k),
                            start=(k == 0),
                            stop=(k == KD - 1),
                        )
                    g_sb = gpool.tile([128, 512], bf16, tag="gact")
                    nc.scalar.activation(
                        g_sb[:, :w],
                        ps_g[:, :w],
                        mybir.ActivationFunctionType.Silu,
                    )
                    nc.vector.tensor_mul(h_sb[:, m, :w], g_sb[:, :w], ps_u[:, :w])
                o_sb = opool.tile([128, KD * w], bf16, tag="o")
                for m2 in range(KD):
                    ps_d = psd.tile([128, 512], f32, tag="psd")
                    for k2 in range(KF):
                        nc.tensor.matmul(
                            ps_d[:, :w],
                            wd_slice(m2, k2),
                            h_sb[:, k2, :w],
                            start=(k2 == 0),
                            stop=(k2 == KF - 1),
                        )
                    nc.vector.tensor_copy(o_sb[:, m2 * w : (m2 + 1) * w], ps_d[:, :w])
                    # last group's outs ride the (idle by then) sync ring so
                    # the final out DMA isn't queued behind scalar traffic
                    out_eng = nc.sync if gi == order[-1] else nc.scalar
                    out_eng.dma_start(
                        out_d.ap()[:, KD * off + m2 * w : KD * off + (m2 + 1) * w],
                        o_sb[:, m2 * w : (m2 + 1) * w],
                    )

    nc.compile()
    return nc


def kernel(hidden_states, router_w, router_b, wg, wu, wd, top_k):
    hidden_states = np.asarray(hidden_states, dtype=np.float32)
    router_w = np.asarray(router_w, dtype=np.float32)
    router_b = np.asarray(router_b, dtype=np.float32)
    wg = np.asarray(wg, dtype=np.float32)
    wu = np.asarray(wu, dtype=np.float32)
    wd = np.asarray(wd, dtype=np.float32)

    Bb, S, Dd = hidden_states.shape
    x = hidden_states.reshape(-1, Dd)
    assert x.shape == (T, D)

    topi, topw = _route(x, router_w, router_b, top_k)
    per_expert = {}
    for e in range(1, E):
        sel = np.nonzero((topi == (e - 1)).any(axis=1))[0]
        w = topw[sel][topi[sel] == (e - 1)]
        per_expert[e] = (sel.astype(np.int64), w.astype(np.float32))

    A, B, slots = _plan_slots(per_expert)
    tiles, total = _tiles_for(A, B)

    # Per-expert weight layouts (bf16, m-tile-major); computed once per expert.
    experts_used = sorted({s[0] for core in slots for s in core})
    wg_l = {e: _wgu_layout(wg[e]) for e in experts_used}
    wu_l = {e: _wgu_layout(wu[e]) for e in experts_used}
    wd_l = {e: _wd_layout(wd[e]) for e in experts_used}

    in_maps = []
    for c in range(NCORES):
        (eA, idxA, _wA), (eB, idxB, _wB) = slots[c]
        xt_flat = np.zeros((128, KD * total), dtype=_BF16)
        for tg_, ts, w, off in tiles:
            idx = (idxA, idxB)[tg_]
            seg = idx[ts : ts + w]
            gx = np.zeros((w, D), dtype=np.float32)
            gx[: len(seg)] = x[seg]
            blk = gx.T.reshape(KD, 128, w).transpose(1, 0, 2).reshape(128, KD * w)
            xt_flat[:, KD * off : KD * (off + w)] = blk.astype(_BF16)
        m = {"xt": xt_flat}
        groups = [g for g in range(2) if (A, B)[g] > 0]
        for g in groups:
            e = (eA, eB)[g]
            m[f"w{g}g"] = wg_l[e]
            m[f"w{g}u"] = wu_l[e]
            m[f"w{g}d"] = wd_l[e]
        in_maps.append(m)

    if (A, B) not in _COMPILED:
        _COMPILED[(A, B)] = _build_graph(A, B)
    nc = _COMPILED[(A, B)]

    # If the environment lacks antenv.axon_hooks, running with BASS_TRACE=1
    # would crash inside run_bass_kernel_spmd on an unguarded import; provide
    # an inert hook registry so tracing degrades to a warning instead.
    try:
        import antenv.axon_hooks  # noqa: F401
    except Exception:
        import sys as _sys
        import types as _types

        _m = _types.ModuleType("antenv.axon_hooks")
        _m._h = None
        _m.set_axon_ntff_profile_hook = lambda h: setattr(_m, "_h", h)
        _m.get_axon_ntff_profile_hook = lambda: getattr(_m, "_h", None)
        _sys.modules["antenv.axon_hooks"] = _m

    from concourse.bass_utils import run_bass_kernel_spmd

    res = run_bass_kernel_spmd(nc, in_maps, core_ids=list(range(NCORES)))
    global LAST_EXEC_NS
    LAST_EXEC_NS = res.exec_time_ns

    out = np.zeros((T, D), dtype=np.float32)
    for c in range(NCORES):
        yT = res.results[c]["out"]  # [128, KD*total] f32
        for tg_, ts, w, off in tiles:
            e, idx, wt = slots[c][tg_]
            seg = idx[ts : ts + w]
            wseg = wt[ts : ts + w]
            if len(seg) == 0:
                continue
            y = (
                yT[:, KD * off : KD * (off + w)]
                .astype(np.float32)
                .reshape(128, KD, w)
                .transpose(1, 0, 2)
                .reshape(D, w)
                .T
            )
            out[seg] += wseg[:, None] * y[: len(seg)]

    return out.reshape(Bb, S, D)



# revision 27
# speedup vs baseline: 1.0104x; 1.0104x over previous
"""Always-on MoE forward (expert 0 dense + top-k of 7 routed) on 8 TRN2 cores.

Strategy
--------
The router (4096x1024 @ 1024x7 matmul + softmax + top-2) is ~58 MFLOP --
negligible -- so it runs on host in numpy as part of computing the sharding
plan.  The expensive part (expert SwiGLU MLPs, ~155 GFLOP with top-2
sparsity) runs on device, expert-parallel with host-side token
dispatch/combine:

- SPMD graph: every core runs TWO weight groups with compile-time capacities
  (A, B).  Each (core, group) slot is filled with tokens of ONE expert
  (weights supplied per-core via in_maps).  The 7 routed experts are packed
  into the 8 B-slots (splitting an expert across slots when that lowers the
  makespan); leftover B-slots and all A-slots are filled with always-on
  expert-0 tokens.  (A, B) are optimized per call from the actual router
  counts, so per-core capacity lands within a few % of the ideal
  (T + k*T) / 8 balance point.
- Host gathers each core's tokens (transposed k-chunk layout, bf16), device
  computes down(silu(x@wg) * (x@wu)) for both groups, host scatter-adds the
  outputs with the combine weights (expert-0 weight 1.0).

Device kernel: pure dense matmul streaming, weights stationary in SBUF per
group.  All DRAM views are pre-tiled on host into [128, flat] layouts so
every DMA is contiguous per partition and every matmul operand is a direct
SBUF slice.

DMA schedule (measured on HW, see comments in _build_graph): geometric
[1,1,2,4,8]-m-tile weight blocks in exact consumption order across the two
HWDGE rings -- sync carries wg + the first two wu blocks + wd, scalar
carries xt (first tile k-chunked) + the remaining wu + outputs.  A ~4 us
PE pre-warm burst flips the HAM clock gate during the DMA ramp so payload
matmuls start warm the moment wg block0 lands (~12 us).  Keep per-ring
dma_start counts low: each dma_start costs its engine ~0.6-1.2 us and only
~6 can be outstanding; finer-grained schedules measured SLOWER (engine
issue serialization starves the ring mid-stream), and the gpsimd SWDGE
queue is far too slow for bulk tiles (+65 us when tried).
"""

import numpy as np
import ml_dtypes

D = 1024
DFF = 2048
E = 8
NCORES = 8
T = 2 * 2048  # B * S
KD = D // 128    # contraction chunks over D
KF = DFF // 128  # contraction chunks over DFF

_COMPILED = {}

_BF16 = ml_dtypes.bfloat16

LAST_EXEC_NS = None


def _route(x, router_w, router_b, top_k):
    """Replicates the reference router in numpy f32: returns (topi, topw)."""
    logits = x.astype(np.float32) @ router_w.astype(np.float32) + router_b.astype(
        np.float32
    )
    m = logits.max(axis=-1, keepdims=True)
    p = np.exp(logits - m)
    p /= p.sum(axis=-1, keepdims=True)
    k = int(top_k)
    topi = np.argpartition(-p, kth=k - 1, axis=-1)[:, :k]  # top-k set (unordered)
    topw = np.take_along_axis(p, topi, axis=-1)
    topw = topw / topw.sum(axis=-1, keepdims=True)
    return topi, topw.astype(np.float32)


def _split_even(idx, w, n):
    """Split (idx, w) into n near-equal chunks."""
    c = len(idx)
    sizes = [(c + n - 1 - i) // n for i in range(n)]
    out, pos = [], 0
    for s in sizes:
        out.append((idx[pos : pos + s], w[pos : pos + s]))
        pos += s
    return out


def _widths_for(cap):
    """Tile widths for a group capacity.

    First tile as wide as possible (512) -- during the first tile the weight
    stream runs near the HBM rate, and a wider tile lowers the per-ns weight
    demand.  Remaining capacity in near-equal tiles, kept above the ~280
    matmul instruction floor (LDWEIGHTS + dispatch) when possible.
    """
    if cap <= 0:
        return []

    def near_equal(c, n):
        return [(c + n - 1 - i) // n for i in range(n)]

    plain = near_equal(cap, -(-cap // 512))
    if cap > 512:
        rem = cap - 512
        lead = [512] + near_equal(rem, -(-rem // 512))
    else:
        lead = plain
    cost = lambda ws: sum(max(w, 280) for w in ws)
    return lead if cost(lead) <= cost(plain) else plain


def _tile_cost(cap):
    return sum(max(w, 280) for w in _widths_for(cap))


def _plan_slots(per_expert):
    """Pick capacities (A, B) and fill 8 A-slots + 8 B-slots.

    per_expert: {e: (idx, w)} for routed experts.
    Returns (A, B, slots) with slots[c] = ((eA, idxA, wA), (eB, idxB, wB)).
    """
    counts = {e: len(v[0]) for e, v in per_expert.items()}
    cands = sorted(
        {-(-c // j) for c in counts.values() for j in range(1, 9) if c} | {512}
    )
    best = None
    for Bc in cands:
        npieces = sum(-(-c // Bc) for c in counts.values() if c)
        if npieces > NCORES:
            continue
        nfree = NCORES - npieces
        e0_in_b = min(T, nfree * Bc)
        A = -(-(T - e0_in_b) // NCORES) if e0_in_b < T else 0
        tot = _tile_cost(A) + _tile_cost(Bc)
        if best is None or tot < best[0]:
            best = (tot, A, Bc)
    _, A, B = best

    ones = np.ones(T, dtype=np.float32)
    e0_idx = np.arange(T, dtype=np.int64)

    b_slots = []
    for e in sorted(counts, key=lambda e: -counts[e]):
        idx, w = per_expert[e]
        if len(idx) == 0:
            continue
        n = -(-len(idx) // B)
        for piece in _split_even(idx, w, n):
            b_slots.append((e, piece[0], piece[1]))
    pos = 0
    while len(b_slots) < NCORES:
        take = min(B, T - pos)
        b_slots.append((0, e0_idx[pos : pos + take], ones[pos : pos + take]))
        pos += take
    rem_idx = e0_idx[pos:]
    a_pieces = _split_even(rem_idx, ones[pos:], NCORES)
    a_slots = [(0, p[0], p[1]) for p in a_pieces]

    slots = [(a_slots[c], b_slots[c]) for c in range(NCORES)]
    return A, B, slots


def _wgu_layout(w2d):
    """[D, DFF] f32 -> [128, KF*KD*128] bf16 m-tile-major layout:
    element [p, (m*KD + k)*128 + c] = W[k*128 + p, m*128 + c]."""
    a = w2d.reshape(KD, 128, KF, 128).transpose(1, 2, 0, 3).reshape(128, -1)
    return np.ascontiguousarray(a).astype(_BF16)


def _wd_layout(w2d):
    """[DFF, D] f32 -> [128, KD*KF*128] bf16 m-tile-major layout:
    element [p, (m*KF + k)*128 + c] = W[k*128 + p, m*128 + c]."""
    a = w2d.reshape(KF, 128, KD, 128).transpose(1, 2, 0, 3).reshape(128, -1)
    return np.ascontiguousarray(a).astype(_BF16)


def _tiles_for(A, B):
    """Token tiles [(group, start_in_group, width, flat_offset)] per core."""
    tiles = []
    off = 0
    for g, cap in enumerate([A, B]):
        ts = 0
        for w in _widths_for(cap):
            tiles.append((g, ts, w, off))
            off += w
            ts += w
    return tiles, off


def _build_graph(A, B):
    import concourse.mybir as mybir
    import concourse.tile as tile
    from concourse import bacc
    from contextlib import ExitStack

    bf16 = mybir.dt.bfloat16
    f32 = mybir.dt.float32

    tiles, total = _tiles_for(A, B)
    groups = [g for g in range(2) if (A, B)[g] > 0]

    nc = bacc.Bacc("TRN2", target_bir_lowering=False)

    xt_d = nc.declare_dram_parameter("xt", [128, KD * total], bf16, isOutput=False)
    w_ds = []
    for g in groups:
        w_ds.append(
            (
                nc.declare_dram_parameter(
                    f"w{g}g", [128, KF * KD * 128], bf16, isOutput=False
                ),
                nc.declare_dram_parameter(
                    f"w{g}u", [128, KF * KD * 128], bf16, isOutput=False
                ),
                nc.declare_dram_parameter(
                    f"w{g}d", [128, KD * KF * 128], bf16, isOutput=False
                ),
            )
        )
    out_d = nc.declare_dram_parameter("out", [128, KD * total], bf16, isOutput=True)

    with tile.TileContext(nc) as tc, ExitStack() as ctx:
        wpool = ctx.enter_context(tc.tile_pool(name="weights", bufs=1))
        xpool = ctx.enter_context(tc.tile_pool(name="x", bufs=3))
        hpool = ctx.enter_context(tc.tile_pool(name="h", bufs=2))
        gpool = ctx.enter_context(tc.tile_pool(name="gact", bufs=3))
        opool = ctx.enter_context(tc.tile_pool(name="o", bufs=2))
        psg = ctx.enter_context(tc.tile_pool(name="psg", bufs=2, space="PSUM"))
        psu = ctx.enter_context(tc.tile_pool(name="psu", bufs=2, space="PSUM"))
        psd = ctx.enter_context(tc.tile_pool(name="psd", bufs=2, space="PSUM"))

        GU_M = KD * 128  # bytes-per-m-tile span (elems) for wg/wu
        D_M = KF * 128   # for wd

        # PE pre-warm: dummy matmuls run while the first weight/token DMAs are
        # in flight, flipping the HAM clock gate to 8/8 (2.4 GHz) before the
        # first real matmul lands.  Sized to end right as the first payload
        # matmul's inputs (wg block0 + xt0 chunk0) land.
        warm_sb = wpool.tile([128, 128], bf16, tag="warm")
        nc.vector.memset(warm_sb[:], 0)
        psw = ctx.enter_context(tc.tile_pool(name="psw", bufs=1, space="PSUM"))
        ps_w = psw.tile([128, 128], f32, tag="psw")
        for _ in range(38):
            nc.tensor.matmul(ps_w[:], warm_sb[:], warm_sb[:], start=True, stop=True)

        # Larger group first: its weights stream in unblocked at t=0, and the
        # other group's weight reloads (WAR-gated on this group's last use of
        # each m-tile slot) get a long compute window to hide under.
        order = sorted(range(len(groups)), key=lambda gi: -((A, B)[groups[gi]]))

        # DMA schedule, tuned against measured ring behavior: ~150-185 GB/s
        # per HWDGE ring when both flow, ~2 us sem-receipt lag per block,
        # each dma_start costs its engine ~0.6-1.2 us of issue time and only
        # ~6 may be outstanding per engine -- so blocks must stay FEW (the
        # geometric [1,1,2,4,8] m-tile split) and in consumption order.
        #   sync ring:   wg_b0, wu_b0, wg_b1, wu_b1, wg_b2..b4, wd_b0..b2
        #   scalar ring: xt0 in 4 k-chunks (first matmul needs only 0.25 MB),
        #                wu_b2..b4, remaining xt tiles, down-phase out DMAs.
        # wd is first needed ~55 us after payload start, so it tails sync.
        GU_BLKS = [(0, 1), (1, 2), (2, 4), (4, 8), (8, 16)]   # m-tile ranges
        WD_BLKS = [(0, 2), (2, 4), (4, 8)]                    # m2-tile ranges

        for gi in order:
            g = groups[gi]
            wg_d, wu_d, wd_d = w_ds[gi]
            first_group = gi == order[0]
            g_tiles = [(ts2, w_, off) for tg2, ts2, w_, off in tiles if tg2 == g]
            xt_sbs = {}
            ts0, w0, off0 = g_tiles[0]
            xt_sb0 = xpool.tile([128, KD * w0], bf16, tag="xt")
            src0 = xt_d.ap()[:, KD * off0 : KD * (off0 + w0)]
            if first_group:
                # xt0 in 4 k-chunks leading the scalar ring: the first
                # matmul needs only the first 0.25 MB.  (Tried and REJECTED,
                # all measured slower: finer front chunks + smaller wg_b0
                # head with warmup 32 -- payload starts earlier than the
                # stream can sustain, stalls move after the start and HAM
                # re-throttles; chunks/xt on gpsimd SWDGE -- +65 us; xt0
                # split across sync -- starves wg/wu.)
                for ci in range(4):
                    c0, c1 = 2 * ci * w0, 2 * (ci + 1) * w0
                    nc.scalar.dma_start(xt_sb0[:, c0:c1], src0[:, c0:c1])
            else:
                nc.scalar.dma_start(xt_sb0[:], src0)
            xt_sbs[ts0] = xt_sb0
            wg_blk, wu_blk, wd_blk = [], [], []
            for bi, (s, e) in enumerate(GU_BLKS):
                span = (e - s) * GU_M
                tg = wpool.tile([128, span], bf16, tag=f"wg_b{bi}")
                nc.sync.dma_start(tg[:], wg_d.ap()[:, s * GU_M : e * GU_M])
                wg_blk.append(tg)
                tu = wpool.tile([128, span], bf16, tag=f"wu_b{bi}")
                eng = nc.sync if bi < 2 else nc.scalar
                eng.dma_start(tu[:], wu_d.ap()[:, s * GU_M : e * GU_M])
                wu_blk.append(tu)
            # remaining xt tiles of this group (behind wu blocks on scalar;
            # needed only when their token tile starts, ~60+ us later)
            for ts2, w_, off in g_tiles[1:]:
                xt_sb = xpool.tile([128, KD * w_], bf16, tag="xt")
                nc.scalar.dma_start(
                    xt_sb[:], xt_d.ap()[:, KD * off : KD * (off + w_)]
                )
                xt_sbs[ts2] = xt_sb
            for bi, (s, e) in enumerate(WD_BLKS):
                span = (e - s) * D_M
                td = wpool.tile([128, span], bf16, tag=f"wd_b{bi}")
                nc.sync.dma_start(td[:], wd_d.ap()[:, s * D_M : e * D_M])
                wd_blk.append(td)

            def gu_slice(blk_list, m, k):
                for bi, (s, e) in enumerate(GU_BLKS):
                    if s <= m < e:
                        base = ((m - s) * KD + k) * 128
                        return blk_list[bi][:, base : base + 128]
                raise AssertionError

            def wd_slice(m2, k2):
                for bi, (s, e) in enumerate(WD_BLKS):
                    if s <= m2 < e:
                        base = ((m2 - s) * KF + k2) * 128
                        return wd_blk[bi][:, base : base + 128]
                raise AssertionError

            for tg_, ts, w, off in tiles:
                if tg_ != g:
                    continue
                rhs = lambda k, t=xt_sbs[ts]: t[:, k * w : k * w + w]
                h_sb = hpool.tile([128, KF, 512], bf16, tag="h")
                for m in range(KF):
                    ps_g = psg.tile([128, 512], f32, tag="psg")
                    ps_u = psu.tile([128, 512], f32, tag="psu")
                    for k in range(KD):
                        nc.tensor.matmul(
                            ps_g[:, :w],
                            gu_slice(wg_blk, m, k),
                            rhs(k),
                            start=(k == 0),
                            stop=(k == KD - 1),
                        )
                    for k in range(KD):
                        nc.tensor.matmul(
                            ps_u[:, :w],
                            gu_slice(wu_blk, m, k),
                            rhs(k),
                            start=(k == 0),
                            stop=(k == KD - 1),
                        )
                    g_sb = gpool.tile([128, 512], bf16, tag="gact")
                    nc.scalar.activation(
                        g_sb[:, :w],
                        ps_g[:, :w],
                        mybir.ActivationFunctionType.Silu,
                    )
                    nc.vector.tensor_mul(h_sb[:, m, :w], g_sb[:, :w], ps_u[:, :w])
                o_sb = opool.tile([128, KD * w], bf16, tag="o")
                for m2 in range(KD):
                    ps_d = psd.tile([128, 512], f32, tag="psd")
                    for k2 in range(KF):
                        nc.tensor.matmul(
                            ps_d[:, :w],
                            wd_slice(m2, k2),
                            h_sb[:, k2, :w],
                            start=(k2 == 0),
                            stop=(k2 == KF - 1),
                        )
                    nc.vector.tensor_copy(o_sb[:, m2 * w : (m2 + 1) * w], ps_d[:, :w])
                    # last group's outs ride the (idle by then) sync ring so
                    # the final out DMA isn't queued behind scalar traffic
                    out_eng = nc.sync if gi == order[-1] else nc.scalar
                    out_eng.dma_start(
                        out_d.ap()[:, KD * off + m2 * w : KD * off + (m2 + 1) * w],
                        o_sb[:, m2 * w : (m2 + 1) * w],
                    )

    nc.compile()
    return nc


def kernel(hidden_states, router_w, router_b, wg, wu, wd, top_k):
    hidden_states = np.asarray(hidden_states, dtype=np.float32)
    router_w = np.asarray(router_w, dtype=np.float32)
    router_b = np.asarray(router_b, dtype=np.float32)
    wg = np.asarray(wg, dtype=np.float32)
    wu = np.asarray(wu, dtype=np.float32)
    wd = np.asarray(wd, dtype=np.float32)

    Bb, S, Dd = hidden_states.shape
    x = hidden_states.reshape(-1, Dd)
    assert x.shape == (T, D)

    topi, topw = _route(x, router_w, router_b, top_k)
    per_expert = {}
    for e in range(1, E):
        sel = np.nonzero((topi == (e - 1)).any(axis=1))[0]
        w = topw[sel][topi[sel] == (e - 1)]
        per_expert[e] = (sel.astype(np.int64), w.astype(np.float32))

    A, B, slots = _plan_slots(per_expert)
    tiles, total = _tiles_for(A, B)

    # Per-expert weight layouts (bf16, m-tile-major); computed once per expert.
    experts_used = sorted({s[0] for core in slots for s in core})
    wg_l = {e: _wgu_layout(wg[e]) for e in experts_used}
    wu_l = {e: _wgu_layout(wu[e]) for e in experts_used}
    wd_l = {e: _wd_layout(wd[e]) for e in experts_used}

    in_maps = []
    for c in range(NCORES):
        (eA, idxA, _wA), (eB, idxB, _wB) = slots[c]
        xt_flat = np.zeros((128, KD * total), dtype=_BF16)
        for tg_, ts, w, off in tiles:
            idx = (idxA, idxB)[tg_]
            seg = idx[ts : ts + w]
            gx = np.zeros((w, D), dtype=np.float32)
            gx[: len(seg)] = x[seg]
            blk = gx.T.reshape(KD, 128, w).transpose(1, 0, 2).reshape(128, KD * w)
            xt_flat[:, KD * off : KD * (off + w)] = blk.astype(_BF16)
        m = {"xt": xt_flat}
        groups = [g for g in range(2) if (A, B)[g] > 0]
        for g in groups:
            e = (eA, eB)[g]
            m[f"w{g}g"] = wg_l[e]
            m[f"w{g}u"] = wu_l[e]
            m[f"w{g}d"] = wd_l[e]
        in_maps.append(m)

    if (A, B) not in _COMPILED:
        _COMPILED[(A, B)] = _build_graph(A, B)
    nc = _COMPILED[(A, B)]

    # If the environment lacks antenv.axon_hooks, running with BASS_TRACE=1
    # would crash inside run_bass_kernel_spmd on an unguarded import; provide
    # an inert hook registry so tracing degrades to a warning instead.
    try:
        import antenv.axon_hooks  # noqa: F401
    except Exception:
        import sys as _sys
        import types as _types

        _m = _types.ModuleType("antenv.axon_hooks")
        _m._h = None
        _m.set_axon_ntff_profile_hook = lambda h: setattr(_m, "_h", h)
        _m.get_axon_ntff_profile_hook = lambda: getattr(_m, "_h", None)
        _sys.modules["antenv.axon_hooks"] = _m

    from concourse.bass_utils import run_bass_kernel_spmd

    res = run_bass_kernel_spmd(nc, in_maps, core_ids=list(range(NCORES)))
    global LAST_EXEC_NS
    LAST_EXEC_NS = res.exec_time_ns

    out = np.zeros((T, D), dtype=np.float32)
    for c in range(NCORES):
        yT = res.results[c]["out"]  # [128, KD*total] f32
        for tg_, ts, w, off in tiles:
            e, idx, wt = slots[c][tg_]
            seg = idx[ts : ts + w]
            wseg = wt[ts : ts + w]
            if len(seg) == 0:
                continue
            y = (
                yT[:, KD * off : KD * (off + w)]
                .astype(np.float32)
                .reshape(128, KD, w)
                .transpose(1, 0, 2)
                .reshape(D, w)
                .T
            )
            out[seg] += wseg[:, None] * y[: len(seg)]

    return out.reshape(Bb, S, D)

